# revision 1
# baseline (speedup 1.0000x reference)
"""Trainium2 Bass kernel for nn_BulkHamiltonian.

Math (derived from the reference, verified numerically):
  For each Bloch wavevector k = (kx, ky):
    phase1 = sqrt(3)*kx              ; K1 = exp(i*phase1)
    phase2 = sqrt(3)/2*kx + 1.5*ky   ; K2 = exp(i*phase2)
  With r11+r22+r33 = 1.5*I and M^-1 = [[0,I],[I,0]] (a row swap), the
  output H[b] (8x8 complex64) is:
    rows 0-3:  [0 | I4]          -- k-INDEPENDENT constant
    rows 4-7:  [L11[b] | L12]    -- the only k-dependent part
  with L11[b] = [[1.5*I2, -A_tr],[-A_bl, 1.5*I2]], L12 the constant
  +-0.2i pattern, -A_tr = -P + iQ, -A_bl = -P - iQ and (c1=cos ph1 etc.)
    P00 = 0.75 + 0.75*c1           Q00 = 0.75*s1
    P01 = P10 = (sqrt3/4)*(1-c1)   Q01 = Q10 = -(sqrt3/4)*s1
    P11 = 0.25 + 0.25*c1 + c2      Q11 = 0.25*s1 + s2
  Only 16 of the 64 floats of rows 4-7 vary per element; the rest is a
  fixed template.

Kernel strategy (pure data parallel, 8 cores x 125000 elements):
  - The device computes rows 4-7 as [N, 64] float32 (complex64
    interleaved); the constant rows 0-3 are filled host-side during the
    gather/unshard step. This halves HBM write traffic.
  - All k inputs are prefetched to SBUF up-front on the gpsimd (SWDGE)
    queue so the two HWDGE rings carry nothing but the big output DMAs
    (interleaving small reads into the write stream measurably degrades
    HBM write throughput).
  - Persistent SBUF out-buffers [128, NB, 64] hold the constant
    template (initialized once, zero-fill split across vector+gpsimd);
    each tile rewrites only the 16 varying columns and DMAs the whole
    contiguous block out (8 MB per DMA, alternating sync/scalar rings).
  - sin/cos: magic-number round + Cody-Waite range reduction into
    [-pi, pi] (DVE), then ScalarE Sin activation (cos via
    add_range_wrap by +pi/2).
"""

import sys
import types

import numpy as np

import concourse.bacc as bacc
import concourse.mybir as mybir
from concourse import bass_utils
from concourse.tile import TileContext


def _ensure_axon_hooks():
    """bass_utils imports antenv.axon_hooks when tracing is requested (e.g.
    BASS_TRACE=1); that module isn't shipped in this image. Provide it,
    backed by the boot helper's ctypes NTFF hook when available."""
    try:
        import antenv.axon_hooks  # noqa: F401
        return
    except ImportError:
        pass
    hook = None
    try:
        from trn_agent_boot.trn_boot import _ntff_profile_via_ctypes

        hook = _ntff_profile_via_ctypes("/opt/axon/libaxon_pjrt.so")
    except Exception:
        hook = None
    mod = types.ModuleType("antenv.axon_hooks")
    mod.get_axon_ntff_profile_hook = lambda: hook
    mod.set_axon_ntff_profile_hook = lambda h: None
    try:
        import antenv

        sys.modules["antenv.axon_hooks"] = mod
        antenv.axon_hooks = mod
    except ImportError:
        sys.modules["antenv.axon_hooks"] = mod


_ensure_axon_hooks()

B_TOTAL = 1_000_000
N_CORES = 8
N_PER_CORE = B_TOTAL // N_CORES  # 125000
NB = 256                         # batch elements per partition (big tiles)
NB1 = 256                        # second buffer width (symmetric)
N_OBUF = 2                       # output template buffers
ROW_F = 64                       # floats per element emitted by the device

F32 = mybir.dt.float32

SQ3 = 1.7320508075688772
ISQ3 = 0.5773502691896258        # 1/sqrt(3)
C34 = 0.4330127018922193         # sqrt(3)/4
PI = 3.141592653589793
PIO2 = 1.5707963267948966
TWOPI = 6.283185307179586
INV2PI = 0.15915494309189535
MAGIC = 12582912.0               # 1.5 * 2**23: float32 round-to-nearest trick

# Cody-Waite split of 2*pi into three float32 constants (c1 + c2 + c3 ~ 2pi,
# products k*c1, k*c2 exact for small integer k)
CW1 = float(np.float32(6.28125))
_r = TWOPI - float(np.float32(6.28125))
_c2bits = np.float32(_r).view(np.uint32) & np.uint32(0xFFFFF000)
CW2 = float(_c2bits.view(np.float32))
CW3 = float(np.float32(_r - float(_c2bits.view(np.float32))))

# float-column (within the 64-float rows-4..7 slab) -> constant value
CONST_COLS = [
    (0, 1.5), (18, 1.5), (36, 1.5), (54, 1.5),     # 1.5*I2 blocks of L11
    (11, 0.2), (25, -0.2), (47, 0.2), (61, -0.2),  # L12 block
]

# varying columns (rows-4..7 slab):
#   -P00 at 4, 32 ; -P01 at 6, 20, 34, 48 ; -P11 at 22, 50
#   +Q00 at 5 ; -Q00 at 33 ; +Q01 at 7, 21 ; -Q01 at 35, 49
#   +Q11 at 23 ; -Q11 at 51

# constant top rows 0..3 of H: [0 | I4]
TOP_CONST = np.zeros((4, 8), dtype=np.complex64)
for _rr in range(4):
    TOP_CONST[_rr, 4 + _rr] = 1.0


def _tiles(n, nb0, nb1):
    """Tile descriptors (start_row, nb_t, buf_idx) covering [0, n).

    Tiles alternate between the big buffer (nb0) and the small buffer
    (nb1). If a remainder exists, one final tile is emitted whose range
    overlaps the previous tile (identical data is written twice, which
    is harmless).
    """
    caps = [nb0, nb1]
    out = []
    pos = 0
    t = 0
    while pos + 128 * caps[t % 2] <= n:
        out.append((pos, caps[t % 2], t % 2))
        pos += 128 * caps[t % 2]
        t += 1
    rem = n - pos
    if rem:
        nb_t = (rem + 127) // 128
        start = n - 128 * nb_t
        assert start >= 0
        bi = t % 2 if nb_t <= caps[t % 2] else 0
        out.append((start, nb_t, bi))
    return out


def build_nc(n=N_PER_CORE, nb=NB, nb1=NB1, enable_asserts=False):
    nc = bacc.Bacc(
        "TRN2",
        target_bir_lowering=False,
        debug=False,
        enable_asserts=enable_asserts,
    )
    k_ap = nc.dram_tensor("k_in", [n, 2], F32, kind="ExternalInput").ap()
    o_ap = nc.dram_tensor("h_out", [n, ROW_F], F32, kind="ExternalOutput").ap()

    tiles = _tiles(n, nb, nb1)
    tot_nb = sum(nbt for _, nbt, _ in tiles)

    obufs = [
        nc.alloc_sbuf_tensor(f"obuf{i}", [128, cap, ROW_F], F32).ap()
        for i, cap in enumerate([nb, nb1])
    ]
    # all k inputs, prefetched once; tile t's slab starts at column off_t
    k_all = nc.alloc_sbuf_tensor("k_all", [128, tot_nb, 2], F32).ap()

    A = mybir.AluOpType
    AF = mybir.ActivationFunctionType

    def init_buf(ob, width):
        # Zero-fill split across vector and scalar (ACT) so the halves run
        # in parallel. gpsimd compute is deliberately NOT used: GpSimd and
        # DVE share SBUF ports, and a concurrent gpsimd memset starves DVE
        # ops on tile 0's critical path. The ACT half uses Sin(0*x) on a
        # uint32 bitcast so the Sin table set is loaded from the start (no
        # mid-stream ACT table switch before the first Sin). All init runs
        # before the output stream starts; big SBUF memsets concurrent with
        # output DMAs measurably starve the DMA reads.
        half = width // 2
        nc.vector.memset(ob[:, :half, :], 0.0)
        zv = ob[:, half:, :].bitcast(mybir.dt.uint32)
        nc.scalar.activation(zv, zv, AF.Sin, bias=0.0, scale=0.0)
        for idx, (col, val) in enumerate(CONST_COLS):
            cv = ob[:, :, col]
            if idx % 2 == 0:
                nc.vector.memset(cv, val)
            else:
                nc.scalar.activation(cv, cv, AF.Copy, bias=val, scale=0.0)

    with TileContext(nc) as tc:
        # prefetch all k tiles on the gpsimd (SWDGE) queue
        off = 0
        offs = []
        for start, nbt, _bi in tiles:
            offs.append(off)
            nc.gpsimd.dma_start(
                k_all[:, off:off + nbt, :],
                k_ap[start:start + 128 * nbt].rearrange("(p n) c -> p n c", p=128),
            )
            off += nbt

        init_buf(obufs[0], nb)
        init_buf(obufs[1], nb1)

        with tc.tile_pool(name="work", bufs=2) as pool:
            for t, (start, nbt, bi) in enumerate(tiles):
                o = obufs[bi]
                rows = 128 * nbt
                dma_eng = nc.sync

                kx = k_all[:, offs[t]:offs[t] + nbt, 0]
                ky = k_all[:, offs[t]:offs[t] + nbt, 1]

                def tile_(tag):
                    return pool.tile([128, nbt], F32, tag=tag, name=tag)

                c1 = tile_("c1"); s1 = tile_("s1"); c2 = tile_("c2"); s2 = tile_("s2")
                v = tile_("v"); w2 = tile_("w2"); w3 = tile_("w3")
                x1 = tile_("x1"); t1 = tile_("t1"); q1 = tile_("q1")
                y1 = tile_("y1"); yc1 = tile_("yc1")
                x2 = tile_("x2"); t2 = tile_("t2"); q2 = tile_("q2")
                y2 = tile_("y2"); yc2 = tile_("yc2")

                # phase1 = sqrt3*kx; range-reduce into [-pi, pi] via
                # round(x/2pi) (magic-number trick) + Cody-Waite cascade.
                nc.vector.tensor_scalar(x1, kx, SQ3, None, A.mult)
                nc.vector.tensor_scalar(t1, x1, INV2PI, MAGIC, A.mult, A.add)
                nc.vector.tensor_scalar(q1, t1, MAGIC, None, A.subtract)
                nc.vector.cody_waite_cascade(y1, x1, q1, CW1, CW2, CW3)
                nc.vector.add_range_wrap(yc1, y1, PIO2, PI, TWOPI)

                # phase2 = 1.5*(kx/sqrt3 + ky)
                nc.vector.scalar_tensor_tensor(v, kx, ISQ3, ky, A.mult, A.add)
                nc.vector.tensor_scalar(x2, v, 1.5, None, A.mult)
                nc.vector.tensor_scalar(t2, x2, INV2PI, MAGIC, A.mult, A.add)
                nc.vector.tensor_scalar(q2, t2, MAGIC, None, A.subtract)
                nc.vector.cody_waite_cascade(y2, x2, q2, CW1, CW2, CW3)
                nc.vector.add_range_wrap(yc2, y2, PIO2, PI, TWOPI)

                nc.scalar.activation(s1, y1, AF.Sin)
                nc.scalar.activation(c1, yc1, AF.Sin)
                nc.scalar.activation(s2, y2, AF.Sin)
                nc.scalar.activation(c2, yc2, AF.Sin)

                # helpers: w3 = -0.25*c1 - 0.25, w2 = 0.25*s1
                nc.vector.tensor_scalar(w3, c1, -0.25, -0.25, A.mult, A.add)
                nc.vector.tensor_scalar(w2, s1, 0.25, None, A.mult)

                # ---- real parts ----
                # -P00 = -0.75 - 0.75*c1  at cols 4, 32
                nc.scalar.activation(o[:, :nbt, 4], c1, AF.Copy, bias=-0.75, scale=-0.75)
                nc.scalar.activation(o[:, :nbt, 32], c1, AF.Copy, bias=-0.75, scale=-0.75)
                # -P01 = C34*c1 - C34  at cols 6, 20, 34, 48
                nc.vector.tensor_scalar(o[:, :nbt, 6], c1, C34, -C34, A.mult, A.add)
                nc.vector.tensor_scalar(o[:, :nbt, 20], c1, C34, -C34, A.mult, A.add)
                nc.vector.tensor_scalar(o[:, :nbt, 34], c1, C34, -C34, A.mult, A.add)
                nc.vector.tensor_scalar(o[:, :nbt, 48], c1, C34, -C34, A.mult, A.add)
                # -P11 = w3 - c2  at cols 22, 50
                nc.vector.tensor_sub(o[:, :nbt, 22], w3, c2)
                nc.vector.tensor_sub(o[:, :nbt, 50], w3, c2)

                # ---- imag parts ----
                # +Q00 = 0.75*s1 at col 5 ; -Q00 at col 33
                nc.scalar.activation(o[:, :nbt, 5], s1, AF.Copy, bias=0.0, scale=0.75)
                nc.scalar.activation(o[:, :nbt, 33], s1, AF.Copy, bias=0.0, scale=-0.75)
                # +Q01 = -C34*s1 at cols 7, 21 ; -Q01 = +C34*s1 at cols 35, 49
                nc.vector.tensor_scalar(o[:, :nbt, 7], s1, -C34, None, A.mult)
                nc.vector.tensor_scalar(o[:, :nbt, 21], s1, -C34, None, A.mult)
                nc.vector.tensor_scalar(o[:, :nbt, 35], s1, C34, None, A.mult)
                nc.vector.tensor_scalar(o[:, :nbt, 49], s1, C34, None, A.mult)
                # +Q11 = w2 + s2 at col 23 ; -Q11 = -w2 - s2 at col 51
                nc.vector.tensor_add(o[:, :nbt, 23], w2, s2)
                nc.vector.scalar_tensor_tensor(o[:, :nbt, 51], w2, -1.0, s2, A.mult, A.subtract)

                dma_eng.dma_start(
                    o_ap[start:start + rows].rearrange("(p n) c -> p n c", p=128),
                    o[:, :nbt, :],
                )
    nc.compile()
    return nc


_CACHE = {}


def _get_nc():
    if "nc" not in _CACHE:
        _CACHE["nc"] = build_nc()
    return _CACHE["nc"]


def run_spmd(k_flat, **kwargs):
    """k_flat: [B_TOTAL, 2] float32. Returns (per-core results, res obj)."""
    shards = np.ascontiguousarray(k_flat).reshape(N_CORES, N_PER_CORE, 2)
    nc = _get_nc()
    in_maps = [{"k_in": shards[i]} for i in range(N_CORES)]
    res = bass_utils.run_bass_kernel_spmd(
        nc, in_maps, core_ids=list(range(N_CORES)), **kwargs
    )
    return [res.results[i]["h_out"] for i in range(N_CORES)], res


def kernel(k):
    k = np.asarray(k, dtype=np.float32).reshape(B_TOTAL, 2)
    shards, _ = run_spmd(k)
    H = np.empty((B_TOTAL, 8, 8), dtype=np.complex64)
    H[:, 0:4, :] = TOP_CONST  # constant [0 | I4] top rows
    for i in range(N_CORES):
        H[i * N_PER_CORE:(i + 1) * N_PER_CORE, 4:8, :] = (
            shards[i].view(np.complex64).reshape(N_PER_CORE, 4, 8)
        )
    return H



# revision 2
# speedup vs baseline: 3.2261x; 3.2261x over previous
"""Trainium2 Bass kernel for nn_BulkHamiltonian.

Math (derived from the reference, verified numerically):
  For each Bloch wavevector k = (kx, ky):
    phase1 = sqrt(3)*kx              ; c1,s1 = cos/sin(phase1)
    phase2 = sqrt(3)/2*kx + 1.5*ky   ; c2,s2 = cos/sin(phase2)
  With r11+r22+r33 = 1.5*I and M^-1 = [[0,I],[I,0]] (a row swap), the
  output H[b] (8x8 complex64) is:
    rows 0-3:  [0 | I4]          -- k-INDEPENDENT constant
    rows 4-7:  [L11[b] | L12]    -- the only k-dependent part
  Of the 64 floats of rows 4-7, only 16 vary per element and those 16
  take just SIX distinct values (up to sign):
    -P00 = -0.75 - 0.75*c1        (cols 4, 32)
    -P01 =  C34*c1 - C34          (cols 6, 20, 34, 48)
    -P11 = -0.25 - 0.25*c1 - c2   (cols 22, 50)
     Q00 =  0.75*s1               (+col 5, -col 33)
     Q01 = -C34*s1                (+cols 7,21, -cols 35,49)
     Q11 =  0.25*s1 + s2          (+col 23, -col 51)
  The device computes and writes these six planes ([6, N] f32 planar,
  3 MB/core instead of the 32 MB/core full rows-4..7 slab); the host
  places them (plus the static template / sign flips) during the
  gather/unshard step.  Device HBM traffic: 1 MB read + 3 MB write.

Per-phase range reduction into [-pi, pi] uses the magic-number round
fused into ACT Copy's internal fp32 FMA:
    q = fl(fl(x*INV2PI + MAGIC) - MAGIC) = round(x/2pi)   (2 ACT Copies)
    y = x - q*2pi                                          (1 DVE stt /
                                                            ln_bwd_dx)
    yc = add_range_wrap(y + pi/2)                          (1 DVE custom)
Single-step f32 reduction is plenty: |phase| <= ~55 so the f32 error is
~3e-6 rad against a 2e-2 relative output gate.  phase2 is computed as
x2' = phase1 + 3*ky = 2*phase2 (one stt off the existing x1) and reduced
with ln_bwd_dx's (dy - xhat*s0 - s1)*scale fusion at scale=0.5.

Kernel structure (pure data parallel, 8 cores x 125000 elements):
  - k prefetched per-tile on the gpsimd (SWDGE) queue; output DMAs own
    the sync (HWDGE) ring.
  - 4 tiles (nbt=244,244,244,245; the last overlaps 56 rows, writing
    identical values twice).  y/sin buffers are plane-packed
    [128, 4, nbt] so all four Sin evaluations run as ONE ACT op
    (amortizes ACT's 352-cycle fixed cost).
  - Output planes [128, 6, nbt] -> one 3D-AP DMA per tile into the
    planar [6, N] DRAM tensor.
  - Op placement hand-balanced between DVE (~1.1-1.6 ns/elem) and ACT
    (~(nbt+352)/1.2 per op).
"""

import sys
import types

import numpy as np

import concourse.bacc as bacc
import concourse.mybir as mybir
from concourse import bass_utils
from concourse.tile import TileContext


def _ensure_axon_hooks():
    """bass_utils imports antenv.axon_hooks when tracing is requested (e.g.
    BASS_TRACE=1); that module isn't shipped in this image. Provide it,
    backed by the boot helper's ctypes NTFF hook when available."""
    try:
        import antenv.axon_hooks  # noqa: F401
        return
    except ImportError:
        pass
    hook = None
    try:
        from trn_agent_boot.trn_boot import _ntff_profile_via_ctypes

        hook = _ntff_profile_via_ctypes("/opt/axon/libaxon_pjrt.so")
    except Exception:
        hook = None
    mod = types.ModuleType("antenv.axon_hooks")
    mod.get_axon_ntff_profile_hook = lambda: hook
    mod.set_axon_ntff_profile_hook = lambda h: None
    try:
        import antenv

        sys.modules["antenv.axon_hooks"] = mod
        antenv.axon_hooks = mod
    except ImportError:
        sys.modules["antenv.axon_hooks"] = mod


_ensure_axon_hooks()

B_TOTAL = 1_000_000
N_CORES = 8
N_PER_CORE = B_TOTAL // N_CORES  # 125000
NPLANES = 6

F32 = mybir.dt.float32

SQ3 = 1.7320508075688772
C34 = 0.4330127018922193         # sqrt(3)/4
PI = 3.141592653589793
PIO2 = 1.5707963267948966
TWOPI = 6.283185307179586
FOURPI = 12.566370614359172
INV2PI = 0.15915494309189535
INV4PI = 0.07957747154594767
MAGIC = 12582912.0               # 1.5 * 2**23: float32 round-to-nearest trick

# tile descriptors: (start_row, nbt).  125000 = 128*976 + 72, so the last
# tile is widened to 245 and overlaps the previous one by 56 rows.
TILES = [(0, 244), (31232, 244), (62464, 244), (93640, 245)]

# constant top rows 0..3 of H: [0 | I4]
TOP_CONST = np.zeros((4, 8), dtype=np.complex64)
for _rr in range(4):
    TOP_CONST[_rr, 4 + _rr] = 1.0

# static float template of rows 4-7 viewed as [4,16] f32 (re/im interleaved)
ROW_TMPL = np.zeros((4, 16), dtype=np.float32)
for _c, _v in [(0, 1.5), (18, 1.5), (36, 1.5), (54, 1.5),
               (11, 0.2), (25, -0.2), (47, 0.2), (61, -0.2)]:
    ROW_TMPL[_c // 16, _c % 16] = _v

# (flat float column in rows-4..7 slab, plane index, sign)
COL_MAP = [
    (4, 0, +1), (32, 0, +1),                            # -P00
    (6, 1, +1), (20, 1, +1), (34, 1, +1), (48, 1, +1),  # -P01
    (22, 2, +1), (50, 2, +1),                           # -P11
    (5, 3, +1), (33, 3, -1),                            # +/-Q00
    (7, 4, +1), (21, 4, +1), (35, 4, -1), (49, 4, -1),  # +/-Q01
    (23, 5, +1), (51, 5, -1),                           # +/-Q11
]


def build_nc(n=N_PER_CORE, enable_asserts=False):
    nc = bacc.Bacc(
        "TRN2",
        target_bir_lowering=False,
        debug=False,
        enable_asserts=enable_asserts,
    )
    k_ap = nc.dram_tensor("k_in", [n, 2], F32, kind="ExternalInput").ap()
    o_ap = nc.dram_tensor("h_out", [NPLANES, n], F32, kind="ExternalOutput").ap()

    A = mybir.AluOpType
    AF = mybir.ActivationFunctionType

    tot_nb = sum(nbt for _, nbt in TILES)
    k_all = nc.alloc_sbuf_tensor("k_all", [128, tot_nb, 2], F32).ap()
    warm = nc.alloc_sbuf_tensor("warm", [128, 1], F32).ap()

    with TileContext(nc) as tc:
        # load the Sin table set up-front so later Copy/Sin ops don't
        # trigger a mid-stream ACT table switch
        nc.vector.memset(warm, 0.0)
        nc.scalar.activation(warm, warm, AF.Sin, bias=0.0, scale=0.0)

        # prefetch all k tiles on the gpsimd (SWDGE) queue
        off = 0
        offs = []
        for start, nbt in TILES:
            offs.append(off)
            nc.gpsimd.dma_start(
                k_all[:, off:off + nbt, :],
                k_ap[start:start + 128 * nbt].rearrange("(p n) c -> p n c", p=128),
            )
            off += nbt

        with tc.tile_pool(name="work", bufs=2) as pool:
            for t, (start, nbt) in enumerate(TILES):
                rows = 128 * nbt
                kx = k_all[:, offs[t]:offs[t] + nbt, 0]
                ky = k_all[:, offs[t]:offs[t] + nbt, 1]

                x1 = pool.tile([128, nbt], F32, tag="x1", name="x1")
                x2 = pool.tile([128, nbt], F32, tag="x2", name="x2")
                t1 = pool.tile([128, nbt], F32, tag="t1", name="t1")
                q1 = pool.tile([128, nbt], F32, tag="q1", name="q1")
                t2 = pool.tile([128, nbt], F32, tag="t2", name="t2")
                q2 = pool.tile([128, nbt], F32, tag="q2", name="q2")
                yp = pool.tile([128, 4, nbt], F32, tag="yp", name="yp")
                sp = pool.tile([128, 4, nbt], F32, tag="sp", name="sp")
                ob = pool.tile([128, NPLANES, nbt], F32, tag="ob", name="ob")

                y1 = yp[:, 0, :]; yc1 = yp[:, 1, :]
                y2 = yp[:, 2, :]; yc2 = yp[:, 3, :]
                s1 = sp[:, 0, :]; c1 = sp[:, 1, :]
                s2 = sp[:, 2, :]; c2 = sp[:, 3, :]

                # ---- phase 1: x1 = sqrt3*kx, q1 = round(x1/2pi) ----
                nc.vector.tensor_scalar(x1, kx, SQ3, None, A.mult)
                nc.scalar.activation(t1, kx, AF.Copy, bias=MAGIC, scale=SQ3 * INV2PI)
                nc.scalar.activation(q1, t1, AF.Copy, bias=-MAGIC, scale=1.0)
                # y1 = x1 - 2pi*q1 ; yc1 = wrap(y1 + pi/2)
                nc.vector.scalar_tensor_tensor(y1, q1, -TWOPI, x1, A.mult, A.add)
                nc.vector.add_range_wrap(yc1, y1, PIO2, PI, TWOPI)

                # ---- phase 2 (doubled): x2 = x1 + 3*ky = 2*phase2 ----
                nc.vector.scalar_tensor_tensor(x2, ky, 3.0, x1, A.mult, A.add)
                nc.scalar.activation(t2, x2, AF.Copy, bias=MAGIC, scale=INV4PI)
                nc.scalar.activation(q2, t2, AF.Copy, bias=-MAGIC, scale=1.0)
                # y2 = (x2 - 4pi*q2)*0.5 ; yc2 = wrap(y2 + pi/2)
                nc.vector.ln_bwd_dx(y2, x2, q2, FOURPI, 0.0, scale=0.5)
                nc.vector.add_range_wrap(yc2, y2, PIO2, PI, TWOPI)

                # ---- all four sin/cos in one ACT op over the packed planes
                nc.scalar.activation(sp[:, 0:4, :], yp[:, 0:4, :], AF.Sin)

                # ---- output planes ----
                # p0 = -P00 = -0.75*c1 - 0.75            (DVE)
                nc.vector.tensor_scalar(ob[:, 0, :], c1, -0.75, -0.75, A.mult, A.add)
                # p1 = -P01 = C34*c1 - C34               (ACT)
                nc.scalar.activation(ob[:, 1, :], c1, AF.Copy, bias=-C34, scale=C34)
                # p2 = -P11 = (-0.25*c1 - c2) - 0.25     (DVE stt + ACT bias)
                nc.vector.scalar_tensor_tensor(t1, c1, -0.25, c2, A.mult, A.subtract)
                nc.scalar.activation(ob[:, 2, :], t1, AF.Copy, bias=-0.25, scale=1.0)
                # p3 = Q00 = 0.75*s1                     (ACT)
                nc.scalar.activation(ob[:, 3, :], s1, AF.Copy, bias=0.0, scale=0.75)
                # p4 = Q01 = -C34*s1                     (DVE)
                nc.vector.tensor_scalar(ob[:, 4, :], s1, -C34, None, A.mult)
                # p5 = Q11 = 0.25*s1 + s2                (DVE)
                nc.vector.scalar_tensor_tensor(ob[:, 5, :], s1, 0.25, s2, A.mult, A.add)

                nc.sync.dma_start(
                    o_ap[:, start:start + rows].rearrange("c (p n) -> p c n", p=128),
                    ob[:, :, :],
                )
    nc.compile()
    return nc


_CACHE = {}


def _get_nc():
    if "nc" not in _CACHE:
        _CACHE["nc"] = build_nc()
    return _CACHE["nc"]


def run_spmd(k_flat, **kwargs):
    """k_flat: [B_TOTAL, 2] float32. Returns (per-core results, res obj)."""
    shards = np.ascontiguousarray(k_flat).reshape(N_CORES, N_PER_CORE, 2)
    nc = _get_nc()
    in_maps = [{"k_in": shards[i]} for i in range(N_CORES)]
    res = bass_utils.run_bass_kernel_spmd(
        nc, in_maps, core_ids=list(range(N_CORES)), **kwargs
    )
    return [res.results[i]["h_out"] for i in range(N_CORES)], res


def kernel(k):
    k = np.asarray(k, dtype=np.float32).reshape(B_TOTAL, 2)
    shards, _ = run_spmd(k)
    # planes[c] over the full batch, in natural element order
    planes = np.concatenate([s.astype(np.float32) for s in shards], axis=1)

    H = np.empty((B_TOTAL, 8, 8), dtype=np.complex64)
    H[:, 0:4, :] = TOP_CONST
    Hf = H[:, 4:8, :].view(np.float32)   # [B, 4, 16]
    Hf[:] = ROW_TMPL
    Hf = Hf.reshape(B_TOTAL, 64)
    neg = {}
    for col, p, sgn in COL_MAP:
        if sgn > 0:
            Hf[:, col] = planes[p]
        else:
            if p not in neg:
                neg[p] = -planes[p]
            Hf[:, col] = neg[p]
    return H


# revision 3
# speedup vs baseline: 3.8677x; 1.1989x over previous
"""Trainium2 Bass kernel for nn_BulkHamiltonian.

Math (derived from the reference, verified numerically):
  For each Bloch wavevector k = (kx, ky):
    phase1 = sqrt(3)*kx              ; c1,s1 = cos/sin(phase1)
    phase2 = sqrt(3)/2*kx + 1.5*ky   ; c2,s2 = cos/sin(phase2)
  With r11+r22+r33 = 1.5*I and M^-1 = [[0,I],[I,0]] (a row swap), the
  output H[b] (8x8 complex64) is:
    rows 0-3:  [0 | I4]          -- k-INDEPENDENT constant
    rows 4-7:  [L11[b] | L12]    -- the only k-dependent part
  Of the 64 floats of rows 4-7, only 16 vary per element and those 16
  take just SIX distinct values (up to sign):
    -P00 = -0.75 - 0.75*c1        (cols 4, 32)
    -P01 =  C34*c1 - C34          (cols 6, 20, 34, 48)
    -P11 = -0.25 - 0.25*c1 - c2   (cols 22, 50)
     Q00 =  0.75*s1               (+col 5, -col 33)
     Q01 = -C34*s1                (+cols 7,21, -cols 35,49)
     Q11 =  0.25*s1 + s2          (+col 23, -col 51)
  The device computes and writes these six planes ([6, N] f32 planar,
  3 MB/core instead of the 32 MB/core full rows-4..7 slab); the host
  places them (plus the static template / sign flips) during the
  gather/unshard step.  Device HBM traffic: 1 MB read + 3 MB write.

Per-phase range reduction into [-pi, pi] uses the magic-number round
fused into ACT Copy's internal fp32 FMA:
    q = fl(fl(x*INV2PI + MAGIC) - MAGIC) = round(x/2pi)   (2 ACT Copies)
    y = x - q*2pi                                          (1 DVE stt /
                                                            ln_bwd_dx)
    yc = add_range_wrap(y + pi/2)                          (1 DVE custom)
Single-step f32 reduction is plenty: |phase| <= ~55 so the f32 error is
~3e-6 rad against a 2e-2 relative output gate.  phase2 is computed as
x2' = phase1 + 3*ky = 2*phase2 (one stt off the existing x1) and reduced
with ln_bwd_dx's (dy - xhat*s0 - s1)*scale fusion at scale=0.5.

Kernel structure (pure data parallel, 8 cores x 125000 elements):
  - k prefetched per-tile on the gpsimd (SWDGE) queue; output DMAs own
    the sync (HWDGE) ring.
  - 4 tiles (nbt=244,244,244,245; the last overlaps 56 rows, writing
    identical values twice).  y/sin buffers are plane-packed
    [128, 4, nbt] so all four Sin evaluations run as ONE ACT op
    (amortizes ACT's 352-cycle fixed cost).
  - Output planes [128, 6, nbt] -> one 3D-AP DMA per tile into the
    planar [6, N] DRAM tensor.
  - Op placement hand-balanced between DVE (~1.1-1.6 ns/elem) and ACT
    (~(nbt+352)/1.2 per op).
"""

import sys
import types

import numpy as np

import concourse.bacc as bacc
import concourse.mybir as mybir
from concourse import bass_utils
from concourse.tile import TileContext


def _ensure_axon_hooks():
    """bass_utils imports antenv.axon_hooks when tracing is requested (e.g.
    BASS_TRACE=1); that module isn't shipped in this image. Provide it,
    backed by the boot helper's ctypes NTFF hook when available."""
    try:
        import antenv.axon_hooks  # noqa: F401
        return
    except ImportError:
        pass
    hook = None
    try:
        from trn_agent_boot.trn_boot import _ntff_profile_via_ctypes

        hook = _ntff_profile_via_ctypes("/opt/axon/libaxon_pjrt.so")
    except Exception:
        hook = None
    mod = types.ModuleType("antenv.axon_hooks")
    mod.get_axon_ntff_profile_hook = lambda: hook
    mod.set_axon_ntff_profile_hook = lambda h: None
    try:
        import antenv

        sys.modules["antenv.axon_hooks"] = mod
        antenv.axon_hooks = mod
    except ImportError:
        sys.modules["antenv.axon_hooks"] = mod


_ensure_axon_hooks()

B_TOTAL = 1_000_000
N_CORES = 8
N_PER_CORE = B_TOTAL // N_CORES  # 125000
NPLANES = 6

F32 = mybir.dt.float32

SQ3 = 1.7320508075688772
C34 = 0.4330127018922193         # sqrt(3)/4
PI = 3.141592653589793
PIO2 = 1.5707963267948966
TWOPI = 6.283185307179586
FOURPI = 12.566370614359172
INV2PI = 0.15915494309189535
INV4PI = 0.07957747154594767
MAGIC = 12582912.0               # 1.5 * 2**23: float32 round-to-nearest trick

# tile descriptors: (start_row, nbt).  125000 = 128*976 + 72, so the last
# tile is widened to 245 and overlaps the previous one by 56 rows.
TILES = [(0, 244), (31232, 244), (62464, 244), (93640, 245)]

# constant top rows 0..3 of H: [0 | I4]
TOP_CONST = np.zeros((4, 8), dtype=np.complex64)
for _rr in range(4):
    TOP_CONST[_rr, 4 + _rr] = 1.0

# static float template of rows 4-7 viewed as [4,16] f32 (re/im interleaved)
ROW_TMPL = np.zeros((4, 16), dtype=np.float32)
for _c, _v in [(0, 1.5), (18, 1.5), (36, 1.5), (54, 1.5),
               (11, 0.2), (25, -0.2), (47, 0.2), (61, -0.2)]:
    ROW_TMPL[_c // 16, _c % 16] = _v

# (flat float column in rows-4..7 slab, plane index, sign)
COL_MAP = [
    (4, 0, +1), (32, 0, +1),                            # -P00
    (6, 1, +1), (20, 1, +1), (34, 1, +1), (48, 1, +1),  # -P01
    (22, 2, +1), (50, 2, +1),                           # -P11
    (5, 3, +1), (33, 3, -1),                            # +/-Q00
    (7, 4, +1), (21, 4, +1), (35, 4, -1), (49, 4, -1),  # +/-Q01
    (23, 5, +1), (51, 5, -1),                           # +/-Q11
]


def build_nc(n=N_PER_CORE, enable_asserts=False):
    nc = bacc.Bacc(
        "TRN2",
        target_bir_lowering=False,
        debug=False,
        enable_asserts=enable_asserts,
    )
    k_ap = nc.dram_tensor("k_in", [n, 2], F32, kind="ExternalInput").ap()
    o_ap = nc.dram_tensor("h_out", [NPLANES, n], F32, kind="ExternalOutput").ap()

    A = mybir.AluOpType
    AF = mybir.ActivationFunctionType

    tot_nb = sum(nbt for _, nbt in TILES)
    k_all = nc.alloc_sbuf_tensor("k_all", [128, tot_nb, 2], F32).ap()
    warm = nc.alloc_sbuf_tensor("warm", [128, 1], F32).ap()
    negm = nc.alloc_sbuf_tensor("negm", [128, 1], F32).ap()

    with TileContext(nc) as tc:
        # prefetch all k tiles on the gpsimd (SWDGE) queue, first thing
        off = 0
        offs = []
        for start, nbt in TILES:
            offs.append(off)
            nc.gpsimd.dma_start(
                k_all[:, off:off + nbt, :],
                k_ap[start:start + 128 * nbt].rearrange("(p n) c -> p n c", p=128),
            )
            off += nbt

        # load the Sin table set up-front so later Copy/Sin ops don't
        # trigger a mid-stream ACT table switch; -MAGIC broadcast const
        # feeds affine_then_add's in1 slot for the fused round.
        nc.vector.memset(warm, 0.0)
        nc.vector.memset(negm, -MAGIC)
        nc.scalar.activation(warm, warm, AF.Sin, bias=0.0, scale=0.0)

        with tc.tile_pool(name="work", bufs=2) as pool:
            for t, (start, nbt) in enumerate(TILES):
                rows = 128 * nbt
                kx = k_all[:, offs[t]:offs[t] + nbt, 0]
                ky = k_all[:, offs[t]:offs[t] + nbt, 1]
                nmb = negm.to_broadcast([128, nbt])

                w2 = pool.tile([128, nbt], F32, tag="w2", name="w2")
                q1 = pool.tile([128, nbt], F32, tag="q1", name="q1")
                q2 = pool.tile([128, nbt], F32, tag="q2", name="q2")
                yp = pool.tile([128, 4, nbt], F32, tag="yp", name="yp")
                sp = pool.tile([128, 4, nbt], F32, tag="sp", name="sp")
                ob = pool.tile([128, NPLANES, nbt], F32, tag="ob", name="ob")

                y1 = yp[:, 0, :]; yc1 = yp[:, 1, :]
                y2 = yp[:, 2, :]; yc2 = yp[:, 3, :]
                s1 = sp[:, 0, :]; c1 = sp[:, 1, :]
                s2 = sp[:, 2, :]; c2 = sp[:, 3, :]

                # phase1 = sqrt3*kx; q1 = round(phase1/2pi) in one fused
                # DVE op: fl(fma(kx, sqrt3/2pi, MAGIC)) - MAGIC
                nc.vector.affine_then_add(q1, kx, nmb, SQ3 * INV2PI, MAGIC)
                # y1 = sqrt3*kx - 2pi*q1 = (kx - q1*(2pi/sqrt3))*sqrt3
                nc.vector.ln_bwd_dx(y1, kx, q1, TWOPI / SQ3, 0.0, scale=SQ3)
                nc.vector.add_range_wrap(yc1, y1, PIO2, PI, TWOPI)

                # w2 = sqrt3*ky + kx = 2*phase2/sqrt3
                nc.vector.scalar_tensor_tensor(w2, ky, SQ3, kx, A.mult, A.add)
                nc.vector.affine_then_add(q2, w2, nmb, SQ3 * INV4PI, MAGIC)
                # y2 = (w2 - q2*(4pi/sqrt3))*(sqrt3/2) = phase2 - 2pi*q2
                nc.vector.ln_bwd_dx(y2, w2, q2, FOURPI / SQ3, 0.0, scale=SQ3 / 2.0)
                nc.vector.add_range_wrap(yc2, y2, PIO2, PI, TWOPI)

                # all four sin/cos in one ACT op over the packed planes
                nc.scalar.activation(sp[:, 0:4, :], yp[:, 0:4, :], AF.Sin)

                # ---- output planes ----
                # p0 = -P00 = -0.75*c1 - 0.75              (ACT)
                nc.scalar.activation(ob[:, 0, :], c1, AF.Copy, bias=-0.75, scale=-0.75)
                # p1 = -P01 = C34*c1 - C34                 (ACT)
                nc.scalar.activation(ob[:, 1, :], c1, AF.Copy, bias=-C34, scale=C34)
                # p2 = -P11 = -(c2 + 0.25*c1 + 0.25)       (DVE ln_bwd_dx)
                nc.vector.ln_bwd_dx(ob[:, 2, :], c2, c1, -0.25, -0.25, scale=-1.0)
                # p3 = Q00 = 0.75*s1                       (ACT)
                nc.scalar.activation(ob[:, 3, :], s1, AF.Copy, bias=0.0, scale=0.75)
                # p4 = Q01 = -C34*s1                       (ACT)
                nc.scalar.activation(ob[:, 4, :], s1, AF.Copy, bias=0.0, scale=-C34)
                # p5 = Q11 = 0.25*s1 + s2                  (DVE)
                nc.vector.scalar_tensor_tensor(ob[:, 5, :], s1, 0.25, s2, A.mult, A.add)

                nc.sync.dma_start(
                    o_ap[:, start:start + rows].rearrange("c (p n) -> p c n", p=128),
                    ob[:, :, :],
                )
    nc.compile()
    return nc


_CACHE = {}


def _get_nc():
    if "nc" not in _CACHE:
        _CACHE["nc"] = build_nc()
    return _CACHE["nc"]


def run_spmd(k_flat, **kwargs):
    """k_flat: [B_TOTAL, 2] float32. Returns (per-core results, res obj)."""
    shards = np.ascontiguousarray(k_flat).reshape(N_CORES, N_PER_CORE, 2)
    nc = _get_nc()
    in_maps = [{"k_in": shards[i]} for i in range(N_CORES)]
    res = bass_utils.run_bass_kernel_spmd(
        nc, in_maps, core_ids=list(range(N_CORES)), **kwargs
    )
    return [res.results[i]["h_out"] for i in range(N_CORES)], res


def kernel(k):
    k = np.asarray(k, dtype=np.float32).reshape(B_TOTAL, 2)
    shards, _ = run_spmd(k)
    # planes[c] over the full batch, in natural element order
    planes = np.concatenate([s.astype(np.float32) for s in shards], axis=1)

    H = np.empty((B_TOTAL, 8, 8), dtype=np.complex64)
    H[:, 0:4, :] = TOP_CONST
    Hf = H[:, 4:8, :].view(np.float32)   # [B, 4, 16]
    Hf[:] = ROW_TMPL
    Hf = Hf.reshape(B_TOTAL, 64)
    neg = {}
    for col, p, sgn in COL_MAP:
        if sgn > 0:
            Hf[:, col] = planes[p]
        else:
            if p not in neg:
                neg[p] = -planes[p]
            Hf[:, col] = neg[p]
    return H
